# revision 1
# baseline (speedup 1.0000x reference)
"""Trainium2 Bass kernel for the quaternion-KDE (de la Vallee Poussin) problem.

Math: out[m] = (KAPPA+1) * mean_n( clip(|qy_m . qx_n|, 0, 1-1e-7)^(2*KAPPA) )
with qy/qx unit quaternions from MRP vectors Y [65536,3], X [4096,3], KAPPA=50.

Identity used: cos(2*arccos(c)/2) == c, so the arccos/cos pair in the
reference cancels and the kernel value is (KAPPA+1) * |dot|^100.

Device strategy (8 NeuronCores, Y sharded 8192 queries/core, X replicated):
  - Host precomputes outer-product features Q2[i] = vec(q qT) in R^16 so that
    dot^2 = <QY2_m, QX2_n>  (a K=16 contraction; removes abs+square on device).
  - Features are split hi/lo into bf16 pairs and stacked to K=48
    (hi*hi + hi*lo + lo*hi) so the PE runs at bf16 rate (1 cyc/row) with
    ~1e-5 absolute precision on s = dot^2.
  - Per 128-query block: 8 matmuls [48,128]x[48,512] -> PSUM s-tile,
    ACT Ln (bias 1e-5 guards rounding-negative s), ACT Exp(scale=50,
    accum_out) which also row-sums s^50 over the 4096 samples for free.
  - DVE scales the per-block [128,1] sums by 51/4096 into an output buffer,
    DMA'd out once at the end.
"""

import numpy as np
import ml_dtypes

KAPPA = 50.0
N_X = 4096
N_Y = 65536
N_CORES = 8
M_PER_CORE = N_Y // N_CORES  # 8192
N_MB = M_PER_CORE // 128     # 64 query blocks per core
MM_N = 512                   # matmul moving free dim (one PSUM bank of fp32)
LN_BIAS = 1e-5               # guard: s can round slightly negative

_BUILD_CACHE = {}


def _quat(r):
    r = r.astype(np.float64)
    rr = np.sum(r * r, axis=-1, keepdims=True)
    w = (1.0 - rr) / (1.0 + rr)
    v = 2.0 * r / (1.0 + rr)
    return np.concatenate([w, v], axis=-1)  # [n, 4]


def _features(r):
    q = _quat(r)  # [n,4] float64
    return (q[:, :, None] * q[:, None, :]).reshape(q.shape[0], 16)


def _hilo(a64):
    a32 = a64.astype(np.float32)
    hi = a32.astype(ml_dtypes.bfloat16)
    lo = (a32 - hi.astype(np.float32)).astype(ml_dtypes.bfloat16)
    return hi, lo


def _build(n_mb, n_free):
    """Build the Bass module (SPMD; same program for every core)."""
    key = (n_mb, n_free)
    if key in _BUILD_CACHE:
        return _BUILD_CACHE[key]
    import concourse.tile as tile
    import concourse.mybir as mybir
    from concourse import bacc

    f32 = mybir.dt.float32
    bf16 = mybir.dt.bfloat16
    AF = mybir.ActivationFunctionType

    nc = bacc.Bacc("TRN2", debug=False, target_bir_lowering=False)
    yT = nc.dram_tensor("yt", [48, n_mb * 128], bf16, kind="ExternalInput")
    xT = nc.dram_tensor("xt", [48, n_free], bf16, kind="ExternalInput")
    out = nc.dram_tensor("o", [128, n_mb], f32, kind="ExternalOutput")

    n_half = n_free // 2
    scale_out = float((KAPPA + 1.0) / n_free)

    with tile.TileContext(nc) as tc:
        with (
            tc.tile_pool(name="single", bufs=1) as single,
            tc.tile_pool(name="psum", bufs=2, space="PSUM") as pp,
            tc.tile_pool(name="upool", bufs=2) as up,
            tc.tile_pool(name="epool", bufs=2) as ep,
            tc.tile_pool(name="accp", bufs=4) as accp,
        ):
            y_sb = single.tile([48, n_mb * 128], bf16)
            x_sb = single.tile([48, n_free], bf16)
            ob = single.tile([128, n_mb], f32)
            ln_bias = single.tile([128, 1], f32)
            nc.vector.memset(ln_bias[:], LN_BIAS)
            nc.sync.dma_start(out=y_sb[:], in_=yT[:])
            nc.sync.dma_start(out=x_sb[:], in_=xT[:])

            for mb in range(n_mb):
                u = up.tile([128, n_free], f32)
                for h in range(2):
                    s = pp.tile([128, n_half], f32)
                    for j in range(n_half // MM_N):
                        c = h * (n_half // MM_N) + j
                        nc.tensor.matmul(
                            s[:, j * MM_N:(j + 1) * MM_N],
                            y_sb[:, mb * 128:(mb + 1) * 128],
                            x_sb[:, c * MM_N:(c + 1) * MM_N],
                            start=True,
                            stop=True,
                        )
                    nc.scalar.activation(
                        u[:, h * n_half:(h + 1) * n_half], s[:], AF.Ln,
                        bias=ln_bias[:],
                    )
                e = ep.tile([128, n_free], bf16)
                acc = accp.tile([128, 1], f32)
                nc.scalar.activation(
                    e[:], u[:], AF.Exp, scale=KAPPA, accum_out=acc[:]
                )
                nc.vector.tensor_scalar_mul(ob[:, mb:mb + 1], acc[:], scale_out)

            nc.sync.dma_start(out=out[:], in_=ob[:])

    # Force Ln+Exp to be served from the single combined ACT table set, so
    # one ACT_TABLE_LOAD is hoisted to the top instead of 2 swaps per block
    # (measured 128 loads x 1.28us = 164us, 24% of kernel time).
    import types as _types
    import bass_rust as _bass_rust
    from concourse.hw_specs import get_activation_tables as _gat

    def _one_set_loads(self):
        # Keep full list (act_func_set_id indexes act_info.json by position)
        # but make the combined set the only one offering Ln/Exp.
        tables = []
        for k, v in _gat(self.m.arch).items():
            if k != "natural_log_exp_and_others":
                v = v - {AF.Ln, AF.Exp}
            tables.append((k, v))
        _bass_rust.insert_act_table_loads(self, tables)

    nc.insert_act_table_loads = _types.MethodType(_one_set_loads, nc)
    nc.compile()
    _BUILD_CACHE[key] = nc
    return nc


def _prep_inputs(X, Y):
    """Host-side O(M+N) feature prep -> per-core input maps."""
    fx = _features(np.asarray(X))          # [4096, 16]
    fy = _features(np.asarray(Y))          # [65536, 16]
    xhi, xlo = _hilo(fx)
    yhi, ylo = _hilo(fy)
    # rhs rows pair with lhsT rows: (hiY,hiX), (hiY,loX), (loY,hiX)
    xT = np.concatenate([xhi.T, xlo.T, xhi.T], axis=0)  # [48, 4096]
    in_maps = []
    for c in range(N_CORES):
        sl = slice(c * M_PER_CORE, (c + 1) * M_PER_CORE)
        yT = np.concatenate([yhi[sl].T, yhi[sl].T, ylo[sl].T], axis=0)  # [48, 8192]
        in_maps.append({
            "yt": np.ascontiguousarray(yT),
            "xt": np.ascontiguousarray(xT),
        })
    return in_maps


def kernel(X, Y, trace=False):
    from concourse.bass_utils import run_bass_kernel_spmd

    in_maps = _prep_inputs(X, Y)
    nc = _build(N_MB, N_X)
    res = run_bass_kernel_spmd(
        nc, in_maps, core_ids=list(range(N_CORES)), trace=trace
    )
    outs = []
    for r in res.results:
        o = r["o"]  # [128, n_mb]; out[m] with m = mb*128 + p lives at o[p, mb]
        outs.append(np.asarray(o).T.reshape(-1))
    full = np.concatenate(outs, axis=0).astype(np.float32)
    if trace:
        return full, res
    return full



# revision 4
# speedup vs baseline: 1.7359x; 1.7359x over previous
"""Trainium2 Bass kernel for the quaternion-KDE (de la Vallee Poussin) problem.

Math: out[m] = (KAPPA+1) * mean_n( clip(|qy_m . qx_n|, 0, 1-1e-7)^(2*KAPPA) )
with qy/qx unit quaternions from MRP vectors Y [65536,3], X [4096,3], KAPPA=50.

Identities used:
  cos(2*arccos(c)/2) == c, so the kernel value is (KAPPA+1) * |dot|^100
  = 51 * exp(-z) with z = -50*ln(s), s = dot^2 = 1 - w.
  z(w) = -50*ln(1-w) is approximated by the weighted-minimax quadratic
  g(w) = C1*w + C2*w^2 (weight (1-w)^50 = the kernel term itself); max
  error on the term exp(-g) vs exp(-z) is ~4e-5, far inside tolerance.

g is a bidegree-(4,4) polynomial in (qy, qx):
  g = C1*w*P + C2*w^2,  w = P - dot^2,  P = |qy|^2 |qx|^2  (=1 on-sphere)
so g = <phi(qy), psi(qx)> for 35-dim symmetric quartic monomial features
(eigendecomposition of the 35x35 form matrix balances magnitudes).
The matmul therefore emits z directly -- no Ln pass on device. Features
are bf16 hi/lo 3-term stacked (hh+hl+lh) to K=105 <= 128, which is free
on the PE (cost is per output column, independent of K).

Device (8 cores, Y sharded 8192 queries/core, X replicated):
  per 128-query block: 2 halves x 4 matmuls [105,128]x[105,512] -> PSUM
  [128,2048], one ACT Exp in-place per half with scale=-1 and
  bias=ln(51/4096) (folds the mean+prefactor), accum_out giving the
  final per-half row sums; a single strided DVE add at the end combines
  halves into the output block, DMA'd out once.
"""

import math
from collections import defaultdict
from itertools import combinations_with_replacement

import ml_dtypes
import numpy as np

KAPPA = 50.0
N_X = 4096
N_Y = 65536
N_CORES = 8
M_PER_CORE = N_Y // N_CORES  # 8192
N_MB = M_PER_CORE // 128     # 64 query blocks per core
MM_N = 512                   # matmul moving free dim (one PSUM bank of fp32)
NF = 105                     # feature rows: 35 quartic eigenfeatures x (hh,hl,lh)
# weighted-minimax quadratic fit of -50*ln(1-w) on w in [0,0.7], weight (1-w)^50
FIT_C1 = 49.98423095
FIT_C2 = 26.23663952

_BUILD_CACHE = {}
_FEAT_CACHE = {}


def _quat(r):
    r = r.astype(np.float64)
    rr = np.sum(r * r, axis=-1, keepdims=True)
    w = (1.0 - rr) / (1.0 + rr)
    v = 2.0 * r / (1.0 + rr)
    return np.concatenate([w, v], axis=-1)  # [n, 4]


def _quartic_form():
    """35x35 symmetric matrix C with m4(qy)^T C m4(qx) = C1*w*P + C2*w^2."""
    def pmul(p1, p2):
        out = defaultdict(float)
        for (a1, b1), c1 in p1.items():
            for (a2, b2), c2 in p2.items():
                a = tuple(u + v for u, v in zip(a1, a2))
                b = tuple(u + v for u, v in zip(b1, b2))
                out[(a, b)] += c1 * c2
        return dict(out)

    def e1(i):
        v = [0, 0, 0, 0]
        v[i] = 1
        return tuple(v)

    def e2(i, j):
        v = [0, 0, 0, 0]
        v[i] += 1
        v[j] += 1
        return tuple(v)

    D = {(e1(i), e1(i)): 1.0 for i in range(4)}                          # dot
    P = {(e2(i, i), e2(j, j)): 1.0 for i in range(4) for j in range(4)}  # |qy|^2|qx|^2
    D2 = pmul(D, D)
    W = dict(P)
    for k, c in D2.items():
        W[k] = W.get(k, 0.0) - c                                         # w = P - dot^2
    F = defaultdict(float)
    for k, c in pmul(W, P).items():
        F[k] += FIT_C1 * c
    for k, c in pmul(W, W).items():
        F[k] += FIT_C2 * c

    basis = []
    seen = set()
    for comb in combinations_with_replacement(range(4), 4):
        v = [0, 0, 0, 0]
        for i in comb:
            v[i] += 1
        t = tuple(v)
        if t not in seen:
            seen.add(t)
            basis.append(t)
    idx = {t: i for i, t in enumerate(basis)}
    C = np.zeros((35, 35))
    for (a, b), c in F.items():
        C[idx[a], idx[b]] += c
    return 0.5 * (C + C.T), basis


def _monomials(q, basis):
    out = np.empty((q.shape[0], len(basis)))
    for j, t in enumerate(basis):
        v = np.ones(q.shape[0])
        for i in range(4):
            if t[i]:
                v = v * q[:, i] ** t[i]
        out[:, j] = v
    return out


def _eig_factors():
    if "VL" not in _FEAT_CACHE:
        C, basis = _quartic_form()
        lam, V = np.linalg.eigh(C)
        _FEAT_CACHE["VL"] = (lam, V, basis)
    return _FEAT_CACHE["VL"]


def _hilo(a64):
    hi = a64.astype(ml_dtypes.bfloat16)
    lo = (a64 - hi.astype(np.float64)).astype(ml_dtypes.bfloat16)
    return hi, lo


def _build(n_mb, n_free):
    """Build the Bass module (SPMD; same program for every core)."""
    key = (n_mb, n_free)
    if key in _BUILD_CACHE:
        return _BUILD_CACHE[key]
    import concourse.tile as tile
    import concourse.mybir as mybir
    from concourse import bacc

    f32 = mybir.dt.float32
    bf16 = mybir.dt.bfloat16
    AF = mybir.ActivationFunctionType

    nc = bacc.Bacc("TRN2", debug=False, target_bir_lowering=False)
    yT = nc.dram_tensor("yt", [NF, n_mb * 128], bf16, kind="ExternalInput")
    xT = nc.dram_tensor("xt", [NF, n_free], bf16, kind="ExternalInput")
    out = nc.dram_tensor("o", [128, n_mb], f32, kind="ExternalOutput")

    n_half = n_free // 2                      # 2048 = 4 PSUM banks
    exp_bias = float(math.log((KAPPA + 1.0) / n_free))

    with tile.TileContext(nc) as tc:
        with (
            tc.tile_pool(name="single", bufs=1) as single,
            tc.tile_pool(name="psum", bufs=2, space="PSUM") as pp,
        ):
            y_sb = single.tile([NF, n_mb * 128], bf16)
            x_sb = single.tile([NF, n_free], bf16)
            acc = single.tile([128, 2 * n_mb], f32)
            ob = single.tile([128, n_mb], f32)
            eb = single.tile([128, 1], f32)
            nc.vector.memset(eb[:], exp_bias)
            # split input DMAs so block 0 does not wait on the full Y/X loads
            nc.sync.dma_start(out=y_sb[:, :128], in_=yT[:, :128])
            nc.sync.dma_start(out=x_sb[:, :n_half], in_=xT[:, :n_half])
            nc.sync.dma_start(out=x_sb[:, n_half:], in_=xT[:, n_half:])
            nc.sync.dma_start(out=y_sb[:, 128:], in_=yT[:, 128:])

            for mb in range(n_mb):
                yblk = y_sb[:, mb * 128:(mb + 1) * 128]
                for h in range(2):
                    s = pp.tile([128, n_half], f32)
                    for j in range(n_half // MM_N):
                        c = h * (n_half // MM_N) + j
                        nc.tensor.matmul(
                            s[:, j * MM_N:(j + 1) * MM_N],
                            yblk,
                            x_sb[:, c * MM_N:(c + 1) * MM_N],
                            start=True,
                            stop=True,
                        )
                    nc.scalar.activation(
                        s[:], s[:], AF.Exp,
                        scale=-1.0, bias=eb[:],
                        accum_out=acc[:, 2 * mb + h:2 * mb + h + 1],
                    )

            # ob[:, mb] = acc[:, 2mb] + acc[:, 2mb+1]  (single strided DVE op)
            nc.vector.scalar_tensor_tensor(
                ob[:],
                acc[:, 0::2],
                1.0,
                acc[:, 1::2],
                op0=mybir.AluOpType.mult,
                op1=mybir.AluOpType.add,
            )
            nc.sync.dma_start(out=out[:], in_=ob[:])

    nc.compile()
    _BUILD_CACHE[key] = nc
    return nc


def _prep_inputs(X, Y):
    """Host-side O(M+N) feature prep -> per-core input maps."""
    lam, V, basis = _eig_factors()
    qx = _quat(np.asarray(X))
    qy = _quat(np.asarray(Y))
    sq = np.sqrt(np.abs(lam))
    phi = (_monomials(qy, basis) @ V) * sq                   # [65536, 35]
    psi = (_monomials(qx, basis) @ V) * (np.sign(lam) * sq)  # [4096, 35]
    yh, yl = _hilo(phi)
    xh, xl = _hilo(psi)
    # rhs rows pair with lhsT rows: (hiY,hiX), (hiY,loX), (loY,hiX)
    xT = np.concatenate([xh.T, xl.T, xh.T], axis=0)          # [105, 4096]
    xT = np.ascontiguousarray(xT)
    in_maps = []
    for c in range(N_CORES):
        sl = slice(c * M_PER_CORE, (c + 1) * M_PER_CORE)
        yT = np.concatenate([yh[sl].T, yh[sl].T, yl[sl].T], axis=0)  # [105, 8192]
        in_maps.append({
            "yt": np.ascontiguousarray(yT),
            "xt": xT,
        })
    return in_maps


def kernel(X, Y, trace=False):
    from concourse.bass_utils import run_bass_kernel_spmd

    in_maps = _prep_inputs(X, Y)
    nc = _build(N_MB, N_X)
    res = run_bass_kernel_spmd(
        nc, in_maps, core_ids=list(range(N_CORES)), trace=trace
    )
    outs = []
    for r in res.results:
        o = r["o"]  # [128, n_mb]; out[m] with m = mb*128 + p lives at o[p, mb]
        outs.append(np.asarray(o).T.reshape(-1))
    full = np.concatenate(outs, axis=0).astype(np.float32)
    if trace:
        return full, res
    return full
